# revision 14
# baseline (speedup 1.0000x reference)
"""Trainium2 Bass kernel for AttnNoProjVal.

Per batch element b (one NeuronCore each, B=8), using the identity
  scores = q k^T = hs M hs^T + (hs u) 1^T + 1 (hs v)^T + bk.bq,
  M = Wk^T Wq (host-folded), u = Wk^T bq, v = Wq^T bk:
the v and constant terms are per-QUERY-column offsets, which cancel exactly
in softmax and are dropped; the u term is a per-KEY offset folded into the
exp bias. The kernel computes one fused projection g^T = M^T hs^T, then
  scoresT[kp,qp] = (g^T)[:,kp] . (hsq^T)[:,qp]
  E = exp(scoresT/32 + bias[kp])    bias = (hs u)/32 - 3 + mask (host-prep)
  out[qp,:] = (E^T [hs | 1]) / colsum -- colsum via an extra N=1 ones column.

vs the fp32r baseline:
- all matmul operands fp16 (hw matmul stride 215ns vs fp32r's 227ns at 512
  moving rows: 187ns fp32r weight loads don't hide behind streaming, 95ns
  fp16 loads do); fp16 end-to-end error ~9e-4 vs the 2e-2 budget (fp8 /
  DoubleRow and truncated-SVD-of-M variants all blow the budget: softmax
  needs ~1e-2 absolute logit precision).
- the ~10% masked key positions are compacted away on host (key dim 2048 ->
  1920 padded), shrinking projection, scores, and attention-value work 1/16.
- every input is host-relaid so each DMA is one flat [128, X] transfer with
  multi-KB contiguous runs: DMA trigger instructions cost ~0.6us on the
  issuing engine and descriptor generation saturates on segmented patterns,
  so few flat DMAs spread over the sync/gpsimd/scalar queues keep the PE
  fed from t~8us with no mid-phase stalls.
"""

import sys

sys.path.insert(0, "/opt/trn_rl_repo")

from contextlib import ExitStack

import numpy as np

import concourse.tile as tile
from concourse import bacc, mybir
from concourse.bass_utils import run_bass_kernel_spmd

B, S, H = 8, 2048, 1024
N_CORES = 8
HC = H // 128   # 8 chunks of the hidden dim
QB = S // 512   # 4 query blocks
F32 = mybir.dt.float32
F16 = mybir.dt.float16

NKC_DEFAULT = 15  # key chunks after mask compaction (padded to 128)

# m column groups: narrow first group so the first oc chain starts early
MG = [(0, 128), (128, 384), (512, 512)]  # (start, width)
MG_OFS = [0]
for _s, _w in MG:
    MG_OFS.append(MG_OFS[-1] + HC * _w)

_CACHED_NC = {}


def _kb_blocks(nk):
    """key-block (offset, width) list for the projection moving dim."""
    kb = [(0, min(128, nk))]
    o = kb[0][1]
    while o < nk:
        w = min(512, nk - o)
        kb.append((o, w))
        o += w
    return kb


def build_nc(nkc=NKC_DEFAULT):
    nk = nkc * 128
    kb = _kb_blocks(nk)
    nc = bacc.Bacc(None, target_bir_lowering=False)

    # all inputs host-relaid to [128 partitions, X] with contiguous rows
    hstq = nc.dram_tensor("hstq", [128, QB * HC * 512], F16, kind="ExternalInput")
    hstk = nc.dram_tensor("hstk", [128, HC * nk], F16, kind="ExternalInput")
    hsbk = nc.dram_tensor("hsbk", [128, nkc * H], F16, kind="ExternalInput")
    mt = nc.dram_tensor("mt", [128, HC * H], F16, kind="ExternalInput")
    mk = nc.dram_tensor("mk", [128, nkc], F32, kind="ExternalInput")
    out = nc.dram_tensor("out", [S, H], F32, kind="ExternalOutput")

    with tile.TileContext(nc) as tc, ExitStack() as whole:
        singles = whole.enter_context(tc.tile_pool(name="singles", bufs=1))
        gt_pool = whole.enter_context(tc.tile_pool(name="gtp", bufs=1))
        hsbk_pool = whole.enter_context(tc.tile_pool(name="hsbkp", bufs=1))
        qcol_pool = whole.enter_context(tc.tile_pool(name="qcolp", bufs=2))

        junk = singles.tile([128, 512], F16, tag="junk", name="junk")
        nc.vector.memset(junk[:], 0.0)
        bias_sb = singles.tile([128, nkc], F32, tag="bias", name="bias_sb")
        ones_sb = singles.tile([128, 1], F16, tag="ones", name="ones_sb")
        nc.vector.memset(ones_sb[:], 1.0)

        # g^T = M^T hs^T over compacted keys; resident for the whole kernel
        gt = [gt_pool.tile([128, nk], F16, tag=f"gt{d}", name=f"gt{d}") for d in range(HC)]
        hsbk_sb = hsbk_pool.tile([128, nkc * H], F16, tag="hsbk", name="hsbk_sb")
        nc.scalar.dma_start(out=hsbk_sb[:, 0:8 * H], in_=hsbk.ap()[:, 0:8 * H])
        nc.scalar.dma_start(out=hsbk_sb[:, 8 * H:nkc * H], in_=hsbk.ap()[:, 8 * H:nkc * H])
        nc.scalar.dma_start(out=bias_sb[:], in_=mk.ap()[:, :])

        # PE warm-up: keep the PE ticking through the initial DMA wait so the
        # HAM clock-gate opens before the first real matmul.
        with tc.tile_pool(name="psw", bufs=1, space="PSUM") as psw:
            pjunk = psw.tile([128, 512], F32, tag="pj", name="pj")
            for _ in range(30):
                nc.tensor.matmul(
                    pjunk[:], lhsT=junk[:, 0:128], rhs=junk[:], start=True, stop=True
                )

        # ---- Phase A: fused projection g^T into SBUF.
        with ExitStack() as pa:
            wt_pool = pa.enter_context(tc.tile_pool(name="wtp", bufs=1))
            psA = pa.enter_context(tc.tile_pool(name="psA", bufs=8, space="PSUM"))

            m_sb = wt_pool.tile([128, HC * H], F16, tag="m", name="m_sb")
            hstk_sb = wt_pool.tile([128, HC * nk], F16, tag="hstk", name="hstk_sb")
            # start-critical, in need-order; hstk on sync, m on gpsimd so the
            # two trigger+transfer streams run concurrently
            o0, w0 = kb[0]
            nc.sync.dma_start(out=hstk_sb[:, 0:HC * w0], in_=hstk.ap()[:, 0:HC * w0])
            for g in range(len(MG)):
                nc.gpsimd.dma_start(
                    out=m_sb[:, MG_OFS[g]:MG_OFS[g + 1]],
                    in_=mt.ap()[:, MG_OFS[g]:MG_OFS[g + 1]],
                )
            for o, w in kb[1:]:
                nc.sync.dma_start(
                    out=hstk_sb[:, HC * o:HC * (o + w)], in_=hstk.ap()[:, HC * o:HC * (o + w)]
                )
            q0 = qcol_pool.tile([128, HC * 512], F16, tag="qcol", name="qcol")
            nc.sync.dma_start(out=q0[:], in_=hstq.ap()[:, 0:HC * 512])

            def m_lhsT(h, oc):
                if oc == 0:
                    g, ocl = 0, 0
                elif oc < 4:
                    g, ocl = 1, oc - 1
                else:
                    g, ocl = 2, oc - 4
                base = MG_OFS[g] + h * MG[g][1] + ocl * 128
                return m_sb[:, base:base + 128]

            for o, w in kb:
                for oc in range(HC):
                    ps = psA.tile([128, 512], F32, tag="psA", name="psa")
                    for h in range(HC):
                        nc.tensor.matmul(
                            ps[:, 0:w],
                            lhsT=m_lhsT(h, oc),
                            rhs=hstk_sb[:, HC * o + h * w:HC * o + (h + 1) * w],
                            start=(h == 0),
                            stop=(h == HC - 1),
                        )
                    nc.scalar.copy(out=gt[oc][:, o:o + w], in_=ps[:, 0:w])

        # ---- Phase B: scores^T -> exp -> attention-value, per 512-wide block
        # of query positions.
        with ExitStack() as pb:
            et_pool = pb.enter_context(tc.tile_pool(name="etp", bufs=1))
            ps_s = pb.enter_context(tc.tile_pool(name="pss", bufs=3, space="PSUM"))
            ps_o = pb.enter_context(tc.tile_pool(name="pso", bufs=2, space="PSUM"))
            ps_n = pb.enter_context(tc.tile_pool(name="psn", bufs=1, space="PSUM"))
            out_pool = pb.enter_context(tc.tile_pool(name="outp", bufs=2))
            r_pool = pb.enter_context(tc.tile_pool(name="rp", bufs=4))

            for qb in range(QB):
                if qb == 0:
                    qcol = q0
                else:
                    qcol = qnext
                if qb + 1 < QB:
                    # prefetch next query block while this one computes
                    qnext = qcol_pool.tile([128, HC * 512], F16, tag="qcol", name="qcol")
                    nc.sync.dma_start(
                        out=qnext[:],
                        in_=hstq.ap()[:, (qb + 1) * HC * 512:(qb + 2) * HC * 512],
                    )
                et = [et_pool.tile([128, 512], F16, tag=f"et{k}", name=f"et{k}") for k in range(nkc)]
                for k in range(nkc):
                    ps = ps_s.tile([128, 512], F32, tag="pss", name="pss")
                    for d in range(HC):
                        nc.tensor.matmul(
                            ps[:],
                            lhsT=gt[d][:, k * 128:(k + 1) * 128],
                            rhs=qcol[:, d * 512:(d + 1) * 512],
                            start=(d == 0),
                            stop=(d == HC - 1),
                        )
                    nc.scalar.activation(
                        out=et[k][:], in_=ps[:],
                        func=mybir.ActivationFunctionType.Exp,
                        scale=1.0 / 32.0,
                        bias=bias_sb[:, k:k + 1],
                    )
                for qs in range(4):
                    po0 = ps_o.tile([128, 512], F32, tag="po0", name="po0")
                    po1 = ps_o.tile([128, 512], F32, tag="po1", name="po1")
                    pn = ps_n.tile([128, 1], F32, tag="pn", name="pn")
                    for k in range(nkc):
                        lw = et[k][:, qs * 128:(qs + 1) * 128]
                        st, sp = (k == 0), (k == nkc - 1)
                        nc.tensor.matmul(
                            po0[:], lhsT=lw, rhs=hsbk_sb[:, k * H:k * H + 512],
                            start=st, stop=sp,
                        )
                        nc.tensor.matmul(
                            po1[:], lhsT=lw, rhs=hsbk_sb[:, k * H + 512:(k + 1) * H],
                            start=st, stop=sp,
                        )
                        nc.tensor.matmul(pn[:], lhsT=lw, rhs=ones_sb[:], start=st, stop=sp)
                    r = r_pool.tile([128, 1], F32, tag="r", name="r")
                    nc.vector.reciprocal(r[:], pn[:, 0:1])
                    ot = out_pool.tile([128, H], F32, tag="ot", name="ot")
                    row = qb * 512 + qs * 128
                    if qb == QB - 1 and qs == 3:
                        # last group: pipeline scale->store per half to shorten
                        # the serial tail
                        nc.vector.tensor_scalar_mul(out=ot[:, 0:512], in0=po0[:], scalar1=r[:])
                        nc.scalar.dma_start(out=out.ap()[row:row + 128, 0:512], in_=ot[:, 0:512])
                        nc.vector.tensor_scalar_mul(out=ot[:, 512:1024], in0=po1[:], scalar1=r[:])
                        nc.scalar.dma_start(out=out.ap()[row:row + 128, 512:1024], in_=ot[:, 512:1024])
                    else:
                        nc.vector.tensor_scalar_mul(out=ot[:, 0:512], in0=po0[:], scalar1=r[:])
                        nc.vector.tensor_scalar_mul(out=ot[:, 512:1024], in0=po1[:], scalar1=r[:])
                        nc.scalar.dma_start(out=out.ap()[row:row + 128, :], in_=ot[:])

    nc.finalize()
    return nc


def prep_inputs(hidden_states, key_padding_mask, Wq_w, Wq_b, Wk_w, Wk_b):
    """Host prep: fold weights, compact masked keys, relay to DMA-flat
    [128, X] layouts. Returns (nkc, in_maps)."""
    hs = np.ascontiguousarray(hidden_states, dtype=np.float32)
    mask = np.asarray(key_padding_mask, dtype=bool)
    wq = np.asarray(Wq_w, dtype=np.float64)
    wk = np.asarray(Wk_w, dtype=np.float64)
    bq = np.asarray(Wq_b, dtype=np.float64)
    m16 = (wk.T @ wq).astype(np.float32).astype(np.float16)     # [h, oc]
    u = (wk.T @ bq).astype(np.float32)                          # [h]
    hsu = hs.reshape(-1, H) @ u                                 # [B*S]
    bias = (hsu.reshape(B, S) / 32.0 - 3.0).astype(np.float32)

    kmax = int((~mask).sum(axis=1).max())
    nkc = max(1, -(-kmax // 128))
    nk = nkc * 128
    kb = _kb_blocks(nk)

    # m relaid: per partition p, column groups g, h-major inside each group
    m3 = m16.reshape(HC, 128, H)                                # [h, p, oc]
    mt_l = np.concatenate(
        [m3[:, :, s:s + w].transpose(1, 0, 2).reshape(128, HC * w) for s, w in MG],
        axis=1,
    )

    in_maps = []
    for b in range(B):
        sel = np.flatnonzero(~mask[b])
        kk = len(sel)
        hs16 = hs[b].astype(np.float16)                         # [s, d]
        hsk = np.zeros((nk, H), np.float16)
        hsk[:kk] = hs16[sel]                                    # compacted keys
        mkb = np.full(nk, -1e30, np.float32)
        mkb[:kk] = bias[b][sel]

        hsTk = np.ascontiguousarray(hsk.T)                      # [d, keys]
        k3 = hsTk.reshape(HC, 128, nk)                          # [h, p, key]
        hstk_l = np.concatenate(
            [k3[:, :, o:o + w].transpose(1, 0, 2).reshape(128, HC * w) for o, w in kb],
            axis=1,
        )
        hsbk_l = hsk.reshape(nkc, 128, H).transpose(1, 0, 2).reshape(128, nkc * H)
        q3 = hs16.T.reshape(HC, 128, S)                         # [h, p, q]
        hstq_l = np.concatenate(
            [q3[:, :, qb * 512:(qb + 1) * 512].transpose(1, 0, 2).reshape(128, HC * 512)
             for qb in range(QB)],
            axis=1,
        )
        in_maps.append({
            "hstq": np.ascontiguousarray(hstq_l),
            "hstk": np.ascontiguousarray(hstk_l),
            "hsbk": np.ascontiguousarray(hsbk_l),
            "mt": mt_l,
            "mk": np.ascontiguousarray(mkb.reshape(nkc, 128).T),
        })
    return nkc, in_maps


def kernel(hidden_states, key_padding_mask, Wq_w, Wq_b, Wk_w, Wk_b):
    nkc, in_maps = prep_inputs(
        hidden_states, key_padding_mask, Wq_w, Wq_b, Wk_w, Wk_b
    )
    nc = _CACHED_NC.get(nkc)
    if nc is None:
        nc = _CACHED_NC[nkc] = build_nc(nkc)

    res = run_bass_kernel_spmd(nc, in_maps, core_ids=list(range(N_CORES)))
    return np.stack([res.results[b]["out"] for b in range(B)]).astype(np.float32)


# revision 17
# speedup vs baseline: 1.0303x; 1.0303x over previous
"""Trainium2 Bass kernel for AttnNoProjVal.

Per batch element b (one NeuronCore each, B=8), using the identity
  scores = q k^T = hs M hs^T + (hs u) 1^T + 1 (hs v)^T + bk.bq,
  M = Wk^T Wq (host-folded), u = Wk^T bq, v = Wq^T bk:
the v and constant terms are per-QUERY-column offsets, which cancel exactly
in softmax and are dropped; the u term is a per-KEY offset folded into the
exp bias. The kernel computes one fused projection g^T = M^T hs^T, then
  scoresT[kp,qp] = (g^T)[:,kp] . (hsq^T)[:,qp]
  E = exp(scoresT/32 + bias[kp])    bias = (hs u)/32 - 3 + mask (host-prep)
  out[qp,:] = (E^T [hs | 1]) / colsum -- colsum via an extra N=1 ones column.

vs the fp32r baseline:
- all matmul operands fp16 (hw matmul stride 215ns vs fp32r's 227ns at 512
  moving rows: 187ns fp32r weight loads don't hide behind streaming, 95ns
  fp16 loads do); fp16 end-to-end error ~9e-4 vs the 2e-2 budget (fp8 /
  DoubleRow and truncated-SVD-of-M variants all blow the budget: softmax
  needs ~1e-2 absolute logit precision).
- the ~10% masked key positions are compacted away on host (key dim 2048 ->
  1920 padded), shrinking projection, scores, and attention-value work 1/16.
- every input is host-relaid so each DMA is one flat [128, X] transfer with
  multi-KB contiguous runs: DMA trigger instructions cost ~0.6us on the
  issuing engine and descriptor generation saturates on segmented patterns,
  so few flat DMAs spread over the sync/gpsimd/scalar queues keep the PE
  fed from t~8us with no mid-phase stalls.
"""

import sys

sys.path.insert(0, "/opt/trn_rl_repo")

from contextlib import ExitStack

import numpy as np

import concourse.tile as tile
from concourse import bacc, mybir
from concourse.bass_utils import run_bass_kernel_spmd

B, S, H = 8, 2048, 1024
N_CORES = 8
HC = H // 128   # 8 chunks of the hidden dim
QB = S // 512   # 4 query blocks
F32 = mybir.dt.float32
F16 = mybir.dt.float16

NKC_DEFAULT = 15  # key chunks after mask compaction (padded to 128)

# m column groups: narrow first group so the first oc chain starts early
MG = [(0, 128), (128, 384), (512, 512)]  # (start, width)
MG_OFS = [0]
for _s, _w in MG:
    MG_OFS.append(MG_OFS[-1] + HC * _w)

_CACHED_NC = {}


def _kb_blocks(nk):
    """key-block (offset, width) list for the projection moving dim."""
    kb = [(0, min(128, nk))]
    o = kb[0][1]
    while o < nk:
        w = min(512, nk - o)
        kb.append((o, w))
        o += w
    return kb


def build_nc(nkc=NKC_DEFAULT):
    nk = nkc * 128
    kb = _kb_blocks(nk)
    nc = bacc.Bacc(None, target_bir_lowering=False)

    # all inputs host-relaid to [128 partitions, X] with contiguous rows
    hstq = nc.dram_tensor("hstq", [128, QB * HC * 512], F16, kind="ExternalInput")
    hstk = nc.dram_tensor("hstk", [128, HC * nk], F16, kind="ExternalInput")
    hsbk = nc.dram_tensor("hsbk", [128, nkc * H], F16, kind="ExternalInput")
    mt = nc.dram_tensor("mt", [128, HC * H], F16, kind="ExternalInput")
    mk = nc.dram_tensor("mk", [128, nkc], F32, kind="ExternalInput")
    out = nc.dram_tensor("out", [S, H], F32, kind="ExternalOutput")

    with tile.TileContext(nc) as tc, ExitStack() as whole:
        singles = whole.enter_context(tc.tile_pool(name="singles", bufs=1))
        gt_pool = whole.enter_context(tc.tile_pool(name="gtp", bufs=1))
        hsbk_pool = whole.enter_context(tc.tile_pool(name="hsbkp", bufs=1))
        qcol_pool = whole.enter_context(tc.tile_pool(name="qcolp", bufs=2))

        junk = singles.tile([128, 512], F16, tag="junk", name="junk")
        nc.vector.memset(junk[:], 0.0)
        bias_sb = singles.tile([128, nkc], F32, tag="bias", name="bias_sb")
        ones_sb = singles.tile([128, 1], F16, tag="ones", name="ones_sb")
        nc.vector.memset(ones_sb[:], 1.0)

        # g^T = M^T hs^T over compacted keys; resident for the whole kernel
        gt = [gt_pool.tile([128, nk], F16, tag=f"gt{d}", name=f"gt{d}") for d in range(HC)]
        hsbk_sb = hsbk_pool.tile([128, nkc * H], F16, tag="hsbk", name="hsbk_sb")

        # PE warm-up: keep the PE ticking through the initial DMA wait so the
        # HAM clock-gate opens before the first real matmul.
        with tc.tile_pool(name="psw", bufs=1, space="PSUM") as psw:
            pjunk = psw.tile([128, 512], F32, tag="pj", name="pj")
            for _ in range(30):
                nc.tensor.matmul(
                    pjunk[:], lhsT=junk[:, 0:128], rhs=junk[:], start=True, stop=True
                )

        # ---- Phase A: fused projection g^T into SBUF.
        with ExitStack() as pa:
            wt_pool = pa.enter_context(tc.tile_pool(name="wtp", bufs=1))
            psA = pa.enter_context(tc.tile_pool(name="psA", bufs=8, space="PSUM"))

            m_sb = wt_pool.tile([128, HC * H], F16, tag="m", name="m_sb")
            hstk_sb = wt_pool.tile([128, HC * nk], F16, tag="hstk", name="hstk_sb")
            # ALL phase-A loads on ONE queue in exact need-order: concurrent
            # queues split HBM bandwidth and starve the critical path. The
            # phase-B loads (hsbk/bias) are emitted later, behind the second
            # key block's gt copy on the scalar queue, so their transfers
            # can't compete during the phase-A ramp.
            o0, w0 = kb[0]
            nc.sync.dma_start(out=hstk_sb[:, 0:HC * w0], in_=hstk.ap()[:, 0:HC * w0])
            for g in range(len(MG)):
                nc.sync.dma_start(
                    out=m_sb[:, MG_OFS[g]:MG_OFS[g + 1]],
                    in_=mt.ap()[:, MG_OFS[g]:MG_OFS[g + 1]],
                )
            for o, w in kb[1:]:
                nc.sync.dma_start(
                    out=hstk_sb[:, HC * o:HC * (o + w)], in_=hstk.ap()[:, HC * o:HC * (o + w)]
                )
            q0 = qcol_pool.tile([128, HC * 512], F16, tag="qcol", name="qcol")
            nc.sync.dma_start(out=q0[:], in_=hstq.ap()[:, 0:HC * 512])

            def m_lhsT(h, oc):
                if oc == 0:
                    g, ocl = 0, 0
                elif oc < 4:
                    g, ocl = 1, oc - 1
                else:
                    g, ocl = 2, oc - 4
                base = MG_OFS[g] + h * MG[g][1] + ocl * 128
                return m_sb[:, base:base + 128]

            for kbi, (o, w) in enumerate(kb):
                for oc in range(HC):
                    ps = psA.tile([128, 512], F32, tag="psA", name="psa")
                    for h in range(HC):
                        nc.tensor.matmul(
                            ps[:, 0:w],
                            lhsT=m_lhsT(h, oc),
                            rhs=hstk_sb[:, HC * o + h * w:HC * o + (h + 1) * w],
                            start=(h == 0),
                            stop=(h == HC - 1),
                        )
                    nc.scalar.copy(out=gt[oc][:, o:o + w], in_=ps[:, 0:w])
                if kbi == 1:
                    # phase-B loads, queued behind this block's copies so the
                    # transfers start only once the phase-A ramp is fed
                    nc.scalar.dma_start(out=hsbk_sb[:, 0:8 * H], in_=hsbk.ap()[:, 0:8 * H])
                    nc.scalar.dma_start(
                        out=hsbk_sb[:, 8 * H:nkc * H], in_=hsbk.ap()[:, 8 * H:nkc * H]
                    )
                    nc.scalar.dma_start(out=bias_sb[:], in_=mk.ap()[:, :])

        # ---- Phase B: scores^T -> exp -> attention-value, per 512-wide block
        # of query positions.
        with ExitStack() as pb:
            et_pool = pb.enter_context(tc.tile_pool(name="etp", bufs=1))
            ps_s = pb.enter_context(tc.tile_pool(name="pss", bufs=3, space="PSUM"))
            ps_o = pb.enter_context(tc.tile_pool(name="pso", bufs=2, space="PSUM"))
            ps_n = pb.enter_context(tc.tile_pool(name="psn", bufs=1, space="PSUM"))
            out_pool = pb.enter_context(tc.tile_pool(name="outp", bufs=2))
            r_pool = pb.enter_context(tc.tile_pool(name="rp", bufs=4))

            for qb in range(QB):
                if qb == 0:
                    qcol = q0
                else:
                    qcol = qnext
                if qb + 1 < QB:
                    # prefetch next query block while this one computes
                    qnext = qcol_pool.tile([128, HC * 512], F16, tag="qcol", name="qcol")
                    nc.sync.dma_start(
                        out=qnext[:],
                        in_=hstq.ap()[:, (qb + 1) * HC * 512:(qb + 2) * HC * 512],
                    )
                et = [et_pool.tile([128, 512], F16, tag=f"et{k}", name=f"et{k}") for k in range(nkc)]
                for k in range(nkc):
                    ps = ps_s.tile([128, 512], F32, tag="pss", name="pss")
                    for d in range(HC):
                        nc.tensor.matmul(
                            ps[:],
                            lhsT=gt[d][:, k * 128:(k + 1) * 128],
                            rhs=qcol[:, d * 512:(d + 1) * 512],
                            start=(d == 0),
                            stop=(d == HC - 1),
                        )
                    nc.scalar.activation(
                        out=et[k][:], in_=ps[:],
                        func=mybir.ActivationFunctionType.Exp,
                        scale=1.0 / 32.0,
                        bias=bias_sb[:, k:k + 1],
                    )
                for qs in range(4):
                    po0 = ps_o.tile([128, 512], F32, tag="po0", name="po0")
                    po1 = ps_o.tile([128, 512], F32, tag="po1", name="po1")
                    pn = ps_n.tile([128, 1], F32, tag="pn", name="pn")
                    for k in range(nkc):
                        lw = et[k][:, qs * 128:(qs + 1) * 128]
                        st, sp = (k == 0), (k == nkc - 1)
                        nc.tensor.matmul(
                            po0[:], lhsT=lw, rhs=hsbk_sb[:, k * H:k * H + 512],
                            start=st, stop=sp,
                        )
                        nc.tensor.matmul(
                            po1[:], lhsT=lw, rhs=hsbk_sb[:, k * H + 512:(k + 1) * H],
                            start=st, stop=sp,
                        )
                        nc.tensor.matmul(pn[:], lhsT=lw, rhs=ones_sb[:], start=st, stop=sp)
                    r = r_pool.tile([128, 1], F32, tag="r", name="r")
                    nc.vector.reciprocal(r[:], pn[:, 0:1])
                    ot = out_pool.tile([128, H], F32, tag="ot", name="ot")
                    row = qb * 512 + qs * 128
                    if qb == QB - 1 and qs == 3:
                        # last group: pipeline scale->store per half to shorten
                        # the serial tail
                        nc.vector.tensor_scalar_mul(out=ot[:, 0:512], in0=po0[:], scalar1=r[:])
                        nc.scalar.dma_start(out=out.ap()[row:row + 128, 0:512], in_=ot[:, 0:512])
                        nc.vector.tensor_scalar_mul(out=ot[:, 512:1024], in0=po1[:], scalar1=r[:])
                        nc.scalar.dma_start(out=out.ap()[row:row + 128, 512:1024], in_=ot[:, 512:1024])
                    else:
                        nc.vector.tensor_scalar_mul(out=ot[:, 0:512], in0=po0[:], scalar1=r[:])
                        nc.vector.tensor_scalar_mul(out=ot[:, 512:1024], in0=po1[:], scalar1=r[:])
                        nc.scalar.dma_start(out=out.ap()[row:row + 128, :], in_=ot[:])

    nc.finalize()
    return nc


def prep_inputs(hidden_states, key_padding_mask, Wq_w, Wq_b, Wk_w, Wk_b):
    """Host prep: fold weights, compact masked keys, relay to DMA-flat
    [128, X] layouts. Returns (nkc, in_maps)."""
    hs = np.ascontiguousarray(hidden_states, dtype=np.float32)
    mask = np.asarray(key_padding_mask, dtype=bool)
    wq = np.asarray(Wq_w, dtype=np.float64)
    wk = np.asarray(Wk_w, dtype=np.float64)
    bq = np.asarray(Wq_b, dtype=np.float64)
    m16 = (wk.T @ wq).astype(np.float32).astype(np.float16)     # [h, oc]
    u = (wk.T @ bq).astype(np.float32)                          # [h]
    hsu = hs.reshape(-1, H) @ u                                 # [B*S]
    bias = (hsu.reshape(B, S) / 32.0 - 3.0).astype(np.float32)

    kmax = int((~mask).sum(axis=1).max())
    nkc = max(1, -(-kmax // 128))
    nk = nkc * 128
    kb = _kb_blocks(nk)

    # m relaid: per partition p, column groups g, h-major inside each group
    m3 = m16.reshape(HC, 128, H)                                # [h, p, oc]
    mt_l = np.concatenate(
        [m3[:, :, s:s + w].transpose(1, 0, 2).reshape(128, HC * w) for s, w in MG],
        axis=1,
    )

    in_maps = []
    for b in range(B):
        sel = np.flatnonzero(~mask[b])
        kk = len(sel)
        hs16 = hs[b].astype(np.float16)                         # [s, d]
        hsk = np.zeros((nk, H), np.float16)
        hsk[:kk] = hs16[sel]                                    # compacted keys
        mkb = np.full(nk, -1e30, np.float32)
        mkb[:kk] = bias[b][sel]

        hsTk = np.ascontiguousarray(hsk.T)                      # [d, keys]
        k3 = hsTk.reshape(HC, 128, nk)                          # [h, p, key]
        hstk_l = np.concatenate(
            [k3[:, :, o:o + w].transpose(1, 0, 2).reshape(128, HC * w) for o, w in kb],
            axis=1,
        )
        hsbk_l = hsk.reshape(nkc, 128, H).transpose(1, 0, 2).reshape(128, nkc * H)
        q3 = hs16.T.reshape(HC, 128, S)                         # [h, p, q]
        hstq_l = np.concatenate(
            [q3[:, :, qb * 512:(qb + 1) * 512].transpose(1, 0, 2).reshape(128, HC * 512)
             for qb in range(QB)],
            axis=1,
        )
        in_maps.append({
            "hstq": np.ascontiguousarray(hstq_l),
            "hstk": np.ascontiguousarray(hstk_l),
            "hsbk": np.ascontiguousarray(hsbk_l),
            "mt": mt_l,
            "mk": np.ascontiguousarray(mkb.reshape(nkc, 128).T),
        })
    return nkc, in_maps


def kernel(hidden_states, key_padding_mask, Wq_w, Wq_b, Wk_w, Wk_b):
    nkc, in_maps = prep_inputs(
        hidden_states, key_padding_mask, Wq_w, Wq_b, Wk_w, Wk_b
    )
    nc = _CACHED_NC.get(nkc)
    if nc is None:
        nc = _CACHED_NC[nkc] = build_nc(nkc)

    res = run_bass_kernel_spmd(nc, in_maps, core_ids=list(range(N_CORES)))
    return np.stack([res.results[b]["out"] for b in range(B)]).astype(np.float32)
